# revision 14
# baseline (speedup 1.0000x reference)
"""Trainium2 Bass kernel for nn_ClassicalSelfAttention (B=4, S=2048, E=1024).

Reference computation (fp32):
    w_qkv = rotation_params.reshape(3E, E); w_out = entangle_params.reshape(E, E)
    qkv = x @ w_qkv.T; q, k, v = split(qkv)
    scores = (q / sqrt(64)) @ k.T          # full-E attention, no heads
    attn = softmax(scores, axis=-1)
    out = (attn @ v) @ w_out.T
    result = sigmoid(out @ gate_w.T) * out

Because this attention has no per-head reshape, the projections collapse by
associativity, with the weight-weight products precomputed on the host:
    scores = x @ M @ x.T,  M  = wq.T @ wk / sqrt(64)   -> q2 = x @ M
    out    = (attn @ x) @ N.T,  N = w_out @ wv
This removes the separate q/k/v projections and the out projection: per core
928 matmuls instead of 1440, and fewer bf16 quantization steps on the
exp-amplified score path.

Sharding: 8 cores = 4 batches x 2 query-halves. Key order is rotated per
query-half so each core's queries are always columns 0:1024 of its xT input
(softmax and attn@x are permutation-invariant in key order).

All matmul operands are bf16 (1 col/cycle on the PE), accumulation in fp32
PSUM. Scores are computed TRANSPOSED ([kj, qi]) so exp() lands directly in
the layout attn@x needs — no PE transposes. Softmax denominators come from an
all-ones stationary matmul (which broadcasts per-query sums across all 128
partitions for free); normalization is deferred past attn@x AND the N
projection (a per-query-column scale commutes with both) and applied on the
final PSUM->SBUF copy before the sigmoid gate.

Layout (feature-major):
    xT [e, s] (stationary for scoresT, moving for q2); xnat [s, e]
    q2T [e', qi] = M.T-proj of xT
    scoresT [kj, qi] = xT.T @ q2T -> exp -> attT [kj, qi] (bf16, unnormalized)
    sums_bcast = ones.T @ attT (PSUM accum over kj); recip = 1/sums
    axT [e, qi] = xnat.T @ attT (unnormalized)
    outT [g, qi] = (N.T-proj of axT) * recip;  gateT = gwT.T-proj of outT
    resultT = sigmoid(gateT) * outT
Host untransposes the per-core [E, 1024] result tiles.
"""

from contextlib import ExitStack

import numpy as np
import ml_dtypes

import concourse.bass as bass
import concourse.tile as tile
from concourse import bacc, mybir
from concourse.bass_utils import run_bass_kernel_spmd

F32 = mybir.dt.float32
BF16 = mybir.dt.bfloat16

P = 128
E = 1024
B = 4
S = 2048
SK = S            # keys per core (full batch sequence)
SQ = S // 2       # queries per core (half)
ET = E // P       # 8 e-tiles
KT = SK // P      # 16 key tiles
NC = 512          # moving-operand chunk (max free dim per PSUM bank)
SQC = SQ // NC    # 2
NCORES = 8


def _build_nc():
    nc = bacc.Bacc("TRN2", target_bir_lowering=False, debug=False,
                   num_devices=NCORES)
    xT = nc.dram_tensor("xT", [E, SK], BF16, kind="ExternalInput").ap()
    xn = nc.dram_tensor("xn", [SK, E], BF16, kind="ExternalInput").ap()
    m2 = nc.dram_tensor("m2", [E, E], BF16, kind="ExternalInput").ap()
    nt = nc.dram_tensor("nt", [E, E], BF16, kind="ExternalInput").ap()
    gwT = nc.dram_tensor("gwT", [E, E], BF16, kind="ExternalInput").ap()
    outT = nc.dram_tensor("outT", [E, SQ], F32, kind="ExternalOutput").ap()

    with tile.TileContext(nc) as tc, ExitStack() as ctx:
        _emit(tc, ctx, xT, xn, m2, nt, gwT, outT)
    nc.compile()
    return nc


def _emit(tc, ctx, xT, xn, m2, nt, gwT, outT):
    nc = tc.nc
    Exp = mybir.ActivationFunctionType.Exp
    Sigmoid = mybir.ActivationFunctionType.Sigmoid

    singles = ctx.enter_context(tc.tile_pool(name="singles", bufs=1))
    ones = singles.tile([P, P], BF16, tag="ones")
    nc.gpsimd.memset(ones[:], 1.0)

    ps_mm = ctx.enter_context(tc.tile_pool(name="ps_mm", bufs=6, space="PSUM"))
    sums_pool = ctx.enter_context(tc.tile_pool(name="ps_sums", bufs=1, space="PSUM"))
    sums_ps = [sums_pool.tile([P, NC], F32, tag=f"sums{sc}", name=f"sums{sc}")
               for sc in range(SQC)]

    # Warm up the PE p-state during the initial DMA wait: the clock ramps
    # 0.65 -> 1.2 -> 2.4 GHz only after ~3us of continuous matmul activity,
    # so burn that time on throwaway ones-matmuls instead of the first real
    # groups. Accumulates into sums_ps[0], which the real sums group later
    # resets with start=True.
    NWARM = 26
    for i in range(NWARM):
        nc.tensor.matmul(sums_ps[0][:, 0:P], ones[:], ones[:],
                         start=(i == 0), stop=(i == NWARM - 1))

    # Pool stacks are LIFO per side. Left, bottom-up: ctx-lifetime pools
    # (singles, nrm, xnat, axt), then xt/q2t (closed after 2a), then m2
    # (closed after q2 phase), then 2c scratch. Right: w2 (ctx-lifetime),
    # then att (closed after attn@x).
    nrm_pool = ctx.enter_context(tc.tile_pool(name="nrm", bufs=1))
    recip = nrm_pool.tile([P, SQ], F32, tag="recip")
    xn_pool = ctx.enter_context(tc.tile_pool(name="xn", bufs=1))
    xnat = [xn_pool.tile([P, E], BF16, tag=f"xn{i}", name=f"xn{i}")
            for i in range(KT)]
    ax_pool = ctx.enter_context(tc.tile_pool(name="axt", bufs=1))

    es_sc = ExitStack()
    xt_pool = es_sc.enter_context(tc.tile_pool(name="xt", bufs=1))
    q2_pool = es_sc.enter_context(tc.tile_pool(name="q2", bufs=1))
    xt = [xt_pool.tile([P, SK], BF16, tag=f"xt{et}", name=f"xt{et}")
          for et in range(ET)]
    q2t = [q2_pool.tile([P, SQ], BF16, tag=f"q2{i}", name=f"q2{i}")
           for i in range(ET)]

    # ---------------- Loads ----------------
    # Two HWDGE queues: weights/xnat on SP (sync), xT on Activation (scalar).
    # Full [P, >=1024] transfers keep per-partition lines >= 2KB.
    es_m2 = ExitStack()
    m2_pool = es_m2.enter_context(tc.tile_pool(name="m2", bufs=1))
    m2t = []
    for et in range(ET):
        t = m2_pool.tile([P, E], BF16, tag=f"m2{et}", name=f"m2{et}")
        nc.sync.dma_start(out=t[:], in_=m2[et * P:(et + 1) * P, :])
        m2t.append(t)
        nc.scalar.dma_start(
            out=xt[et][:, 0:SQ], in_=xT[et * P:(et + 1) * P, 0:SQ])
    for et in range(ET):
        nc.scalar.dma_start(
            out=xt[et][:, SQ:SK], in_=xT[et * P:(et + 1) * P, SQ:SK])
    for kj in range(KT):
        nc.sync.dma_start(out=xnat[kj][:], in_=xn[kj * P:(kj + 1) * P, :])

    # ---------------- Phase 1: q2T[e', qi] = M-proj of xT ----------------
    for sc in range(SQC):
        for fh in range(4):
            psums = [ps_mm.tile([P, NC], F32, tag="mm", name="mmp")
                     for _ in range(2)]
            for et in range(ET):
                for f2 in range(2):
                    ft = fh * 2 + f2
                    nc.tensor.matmul(
                        psums[f2][:],
                        m2t[et][:, ft * P:(ft + 1) * P],
                        xt[et][:, sc * NC:(sc + 1) * NC],
                        start=(et == 0), stop=(et == ET - 1),
                    )
            for f2 in range(2):
                ft = fh * 2 + f2
                nc.vector.tensor_copy(
                    out=q2t[ft][:, sc * NC:(sc + 1) * NC], in_=psums[f2][:])
    es_m2.close()

    # prefetch 2c weights on the sync queue while scores run
    w2_es = ExitStack()
    w2_pool = w2_es.enter_context(tc.tile_pool(name="wp2", bufs=1, side="right"))
    ntt, gwt = [], []
    for et in range(ET):
        t = w2_pool.tile([P, E], BF16, tag=f"nt{et}", name=f"nt{et}")
        nc.sync.dma_start(out=t[:], in_=nt[et * P:(et + 1) * P, :])
        ntt.append(t)
    for et in range(ET):
        t = w2_pool.tile([P, E], BF16, tag=f"gw{et}", name=f"gw{et}")
        nc.sync.dma_start(out=t[:], in_=gwT[et * P:(et + 1) * P, :])
        gwt.append(t)

    # ---------------- Phase 2a: scoresT -> exp -> attT; sums via ones ----------------
    es_att = ExitStack()
    att_pool = es_att.enter_context(tc.tile_pool(name="att", bufs=1, side="right"))
    att = [att_pool.tile([P, SQ], BF16, tag=f"at{i}", name=f"at{i}")
           for i in range(KT)]

    def emit_sums(kj):
        # per-query exp-sums, broadcast to all 128 partitions by the all-ones
        # stationary; PSUM-accumulated across all 16 key tiles.
        for sc in range(SQC):
            nc.tensor.matmul(
                sums_ps[sc][:],
                ones[:],
                att[kj][:, sc * NC:(sc + 1) * NC],
                start=(kj == 0), stop=(kj == KT - 1),
            )

    for kj in range(KT):
        psums = [ps_mm.tile([P, NC], F32, tag="mm", name="mmp")
                 for _ in range(SQC)]
        for et in range(ET):
            for sc in range(SQC):
                nc.tensor.matmul(
                    psums[sc][:],
                    xt[et][:, kj * P:(kj + 1) * P],
                    q2t[et][:, sc * NC:(sc + 1) * NC],
                    start=(et == 0), stop=(et == ET - 1),
                )
        # sums for the PREVIOUS kj: its exp() ran while this group's
        # matmuls were executing, so the PE never waits on the scalar engine.
        if kj > 0:
            emit_sums(kj - 1)
        for sc in range(SQC):
            nc.scalar.activation(
                out=att[kj][:, sc * NC:(sc + 1) * NC],
                in_=psums[sc][:], func=Exp,
            )
    emit_sums(KT - 1)
    for sc in range(SQC):
        nc.vector.reciprocal(out=recip[:, sc * NC:(sc + 1) * NC],
                             in_=sums_ps[sc][:])

    es_sc.close()  # xt/q2t freed after scores

    # ---------------- Phase 2b: axT[e, qi] = xnat.T @ attT (unnormalized) ----------------
    axt = [ax_pool.tile([P, SQ], BF16, tag=f"ax{i}", name=f"ax{i}")
           for i in range(ET)]
    for et in range(ET):
        psums = [ps_mm.tile([P, NC], F32, tag="mm", name="mmp")
                 for _ in range(SQC)]
        for kj in range(KT):
            for sc in range(SQC):
                nc.tensor.matmul(
                    psums[sc][:],
                    xnat[kj][:, et * P:(et + 1) * P],
                    att[kj][:, sc * NC:(sc + 1) * NC],
                    start=(kj == 0), stop=(kj == KT - 1),
                )
        for sc in range(SQC):
            nc.vector.tensor_copy(
                out=axt[et][:, sc * NC:(sc + 1) * NC], in_=psums[sc][:])

    es_att.close()  # att freed after attn@x

    # ---------------- Phase 2c: outT = (N-proj of axT) * recip; gate; result ----------------
    with tc.tile_pool(name="ot", bufs=1) as ot_pool, \
         tc.tile_pool(name="fin", bufs=2) as fin_pool:

        ot = [ot_pool.tile([P, SQ], BF16, tag=f"ot{i}", name=f"ot{i}")
              for i in range(ET)]
        otf = [ot_pool.tile([P, SQ], F32, tag=f"otf{i}", name=f"otf{i}")
               for i in range(ET)]
        for g in range(ET):
            psums = [ps_mm.tile([P, NC], F32, tag="mm", name="mmp")
                     for _ in range(SQC)]
            for et in range(ET):
                for sc in range(SQC):
                    nc.tensor.matmul(
                        psums[sc][:],
                        ntt[et][:, g * P:(g + 1) * P],
                        axt[et][:, sc * NC:(sc + 1) * NC],
                        start=(et == 0), stop=(et == ET - 1),
                    )
            for sc in range(SQC):
                nc.vector.tensor_mul(
                    ot[g][:, sc * NC:(sc + 1) * NC],
                    psums[sc][:],
                    recip[:, sc * NC:(sc + 1) * NC],
                )
                nc.vector.tensor_mul(
                    otf[g][:, sc * NC:(sc + 1) * NC],
                    psums[sc][:],
                    recip[:, sc * NC:(sc + 1) * NC],
                )

        for g in range(ET):
            psums = [ps_mm.tile([P, NC], F32, tag="mm", name="mmp")
                     for _ in range(SQC)]
            for et in range(ET):
                for sc in range(SQC):
                    nc.tensor.matmul(
                        psums[sc][:],
                        gwt[et][:, g * P:(g + 1) * P],
                        ot[et][:, sc * NC:(sc + 1) * NC],
                        start=(et == 0), stop=(et == ET - 1),
                    )
            fin = fin_pool.tile([P, SQ], F32, tag="fin")
            for sc in range(SQC):
                gate = fin_pool.tile([P, NC], F32, tag="gate")
                nc.scalar.activation(
                    out=gate[:], in_=psums[sc][:], func=Sigmoid)
                nc.vector.tensor_mul(
                    fin[:, sc * NC:(sc + 1) * NC], gate[:],
                    otf[g][:, sc * NC:(sc + 1) * NC])
                # per-chunk output DMA so the final transfer overlaps the
                # last sigmoid/mul instead of trailing the whole group
                nc.sync.dma_start(
                    out=outT[g * P:(g + 1) * P, sc * NC:(sc + 1) * NC],
                    in_=fin[:, sc * NC:(sc + 1) * NC])

    w2_es.close()


_NC_CACHE = None


def _get_nc():
    global _NC_CACHE
    if _NC_CACHE is None:
        _NC_CACHE = _build_nc()
    return _NC_CACHE


def _prep_in_maps(rotation_params, entangle_params, inputs, gate_w):
    w_qkv = np.asarray(rotation_params, dtype=np.float32).reshape(3 * E, E)
    wq, wk, wv = w_qkv[:E], w_qkv[E:2 * E], w_qkv[2 * E:]
    w_out = np.asarray(entangle_params, dtype=np.float32).reshape(E, E)
    gw = np.asarray(gate_w, dtype=np.float32)
    x = np.asarray(inputs, dtype=np.float32)

    bf = ml_dtypes.bfloat16
    # scores = x @ M @ x.T (scale folded); out = (attn @ x) @ N.T
    m2 = ((wq.T @ wk) * np.float32(1.0 / 8.0)).astype(bf)   # [in e, out e']
    ntm = np.ascontiguousarray((w_out @ wv).T).astype(bf)   # [in e, out g]
    gwT = np.ascontiguousarray(gw.T).astype(bf)

    in_maps = []
    for c in range(NCORES):
        b, h = c // 2, c % 2
        xb = x[b]
        if h == 1:   # rotate keys so this core's queries sit at rows 0:SQ
            xb = np.concatenate([xb[SQ:], xb[:SQ]], axis=0)
        in_maps.append({
            "xT": np.ascontiguousarray(xb.T).astype(bf),
            "xn": np.ascontiguousarray(xb).astype(bf),
            "m2": m2, "nt": ntm, "gwT": gwT,
        })
    return in_maps


def _assemble(results):
    out = np.empty((B, S, E), dtype=np.float32)
    for c in range(NCORES):
        b, h = c // 2, c % 2
        out[b, h * SQ:(h + 1) * SQ, :] = results[c]["outT"].T
    return out


def _run(in_maps, trace=False):
    nc = _get_nc()
    return run_bass_kernel_spmd(nc, in_maps, core_ids=list(range(NCORES)),
                                trace=trace)


def kernel(rotation_params, entangle_params, inputs, gate_w):
    in_maps = _prep_in_maps(rotation_params, entangle_params, inputs, gate_w)
    res = _run(in_maps, trace=False)
    return _assemble(res.results)
